# revision 39
# baseline (speedup 1.0000x reference)
"""Causal multi-head attention (QKV proj + 16-head causal attention) on 8 TRN2 cores.

Problem: x [4, 2048, 1024], W [3072, 1024], b [3072] -> out [4, 2048, 1024].
H=16 heads, D=64. Sharding: core c = (batch b = c // 2, head-group g = c % 2);
each core computes batch b, heads g*8 .. g*8+8, producing out[b][:, g*512:(g+1)*512].
No cross-core communication needed.

Device kernel (per core), all matmuls bf16 with f32 PSUM accumulation.
TensorE (~215us busy) and ScalarE (exp over the ~18M live logits, ~158us)
are the co-bottlenecks; the kernel software-pipelines them:
  - QKV projection chunked by 512-token groups. Q/K bias folded into the
    PSUM->SBUF cast via per-partition tensor_scalar_add; V bias added during
    the V cast via a tensor_tensor add against a GpSimd partition_broadcast
    of the bias row (no bias matmuls at all). V lands via one strided CAST
    per t-tile in vA [128, 16, 8, 65]; only vA's ones columns (softmax-
    denominator trick) are memset, keeping GpSimd startup off the
    critical path. Ten identity transposes at t~8us warm the PE clock (HAM)
    so the first projection matmuls run at 2.4 GHz.
  - Attention per (tq-chunk J of 512, head pair hp): S^T tiles [tk=128, tq]
    = kT.T @ qT with the head dim contracting on 64 partitions and both heads
    of the pair issued adjacently (concurrent sub-array execution); exp on
    ScalarE over 2-slot PSUM groups trimmed to the causally-live column
    range (saves ~30us of ScalarE vs full-width); a single 128x128
    upper-triangular mask multiply (on GpSimd, otherwise idle) for diagonal
    tiles only.
  - P@v v-stationary: y^T[65, tq] = sum_i [v_i|1].T @ P^T_i (row 64 = softmax
    denominator). Both heads' y^T pack into one [128, 512] tile via
    partition-shifted DVE casts, then 4 128x128 PE transposes per head pair
    restore token-major layout. Denominators of a head-pair PAIR collect on
    partitions {0,32,64,96} of one [97, 512] tile (DVE partition bases must
    be 32-aligned; memset first so the transposes never touch uninitialized
    bits) -> 4 [97,128] transposes + 4 reciprocals per TWO head pairs, then
    per-partition tensor_scalar_mul normalization straight out of transpose
    PSUM into the output staging tiles.
  - Emission keeps S^T/exp THREE head-pair phases ahead of P@v so ScalarE
    never starves and P@v never waits on exp, with QKV chunk J+1 emitted
    inside attention chunk J. P^T tiles live in a 48-deep pool of per-i-tile
    [128, 2, 512] buffers whose rotation order is deadlock-free at this
    lookahead by construction. Output stages and stores in bf16 (error
    budget 6x headroom), halving output DMA traffic, with the four chunk
    stores split across the Sync and GpSimd DMA queues.
    Input DMAs keep the [512-row] -> [128, 4, *] shape: that split maps
    src row = ct*128 + p; other shapes pair dims differently (verified).
Measured: ~242-244 us NEFF exec (baseline 324 us), rel err 3.4e-3.
"""

import numpy as np
import ml_dtypes

B, T, C = 4, 2048, 1024
H, D = 16, 64
HPC = 8            # heads per core
OC = HPC * D       # 512 output cols per core
NCORES = 8

_cache = {}


def _build_bass():
    import concourse.mybir as mybir
    import concourse.tile as tile
    from concourse import bacc
    from concourse.masks import make_identity, make_upper_triangular

    f32 = mybir.dt.float32
    bf16 = mybir.dt.bfloat16
    EXP = mybir.ActivationFunctionType.Exp

    nc = bacc.Bacc(None)
    xt_d = nc.declare_dram_parameter("xt", [C, T], bf16, isOutput=False)
    wt_d = nc.declare_dram_parameter("wt", [C, 3 * OC], bf16, isOutput=False)
    bcc_d = nc.declare_dram_parameter("bcc", [128, 8], f32, isOutput=False)
    btr_d = nc.declare_dram_parameter("btr", [1, OC], bf16, isOutput=False)
    out_d = nc.declare_dram_parameter("out", [T, OC], bf16, isOutput=True)

    CT = C // 128     # 8 c-tiles
    TT = T // 128     # 16 t-tiles
    TJ = T // 512     # 4 big t-chunks

    with tile.TileContext(nc) as tc:
        with (
            tc.tile_pool(name="persist", bufs=1) as persist,
            tc.tile_pool(name="xtp", bufs=2) as xtp,
            tc.tile_pool(name="qtp", bufs=2) as qtp,
            tc.tile_pool(name="ptp", bufs=48) as ptp,
            tc.tile_pool(name="ytp", bufs=2) as ytp,
            tc.tile_pool(name="denp", bufs=2) as denp,
            tc.tile_pool(name="rcp", bufs=2) as rcp,
            tc.tile_pool(name="osbp", bufs=4) as osbp,
            tc.tile_pool(name="spsum", bufs=2, space="PSUM") as spsum,
            tc.tile_pool(name="shpsum", bufs=2, space="PSUM") as shpsum,
            tc.tile_pool(name="tpsum", bufs=2, space="PSUM") as tpsum,
        ):
            # ---- persistent SBUF tensors ----
            wt = persist.tile([128, CT, 3 * OC], bf16)     # [c%128, c//128, o]
            kT = persist.tile([128, OC // 128, T], bf16)   # [o%128, o//128, t]
            vA = persist.tile([128, TT, HPC, D + 1], bf16)  # v + ones col
            bcc = persist.tile([128, 8], f32)              # Q/K bias, col=o-tile
            btr = persist.tile([1, OC], bf16)              # V bias row
            btrB = persist.tile([128, OC], bf16)           # V bias bcast to all p
            ut = persist.tile([128, 128], bf16)            # upper-tri (incl diag)
            iden = persist.tile([128, 128], bf16)

            # early DMAs: first Q/K weight block + tokens chunk 0 + Q/K bias
            nc.sync.dma_start(wt[:, 0:4, 0:256], wt_d[0:512, 0:256])
            nc.gpsimd.dma_start(wt[:, 4:8, 0:256], wt_d[512:1024, 0:256])

            make_identity(nc, iden[:, :])
            # PE clock (HAM) warmup: identity transposes keep the PE busy from
            # ~1us so the first real matmuls run at 2.4 GHz instead of 1.2
            warm = tpsum.tile([128, 4, 128], bf16, name="warm", tag="tps")
            for k in range(10):
                nc.tensor.transpose(warm[:, k % 4, :], iden[:, :], iden[:, :])
            make_upper_triangular(nc, ut[:, :], val=1.0, diag=True)
            for h in range(HPC):                           # vA ones columns
                nc.gpsimd.memset(vA[:, :, h, D:D + 1], 1.0)

            xts = [None] * TJ
            qts = [None] * TJ
            osbs = {}
            pts = {}

            def load_chunk(tj):
                xts[tj] = xtp.tile([128, CT, 512], bf16, name=f"xt{tj}", tag="xt")
                nc.sync.dma_start(xts[tj][:, 0:4, :],
                                  xt_d[0:512, tj * 512:(tj + 1) * 512])
                nc.gpsimd.dma_start(xts[tj][:, 4:8, :],
                                  xt_d[512:1024, tj * 512:(tj + 1) * 512])
                qts[tj] = qtp.tile([128, 4, 512], bf16, name=f"qt{tj}", tag="qt")

            dens = {}

            def qk_od(tj, g):
                """Project q and k o-tile g for token chunk tj."""
                xtt, qtt = xts[tj], qts[tj]
                for which in range(2):                     # 0 = q, 1 = k
                    ps = shpsum.tile([128, 512], f32, name="ps", tag="ps")
                    w0 = g * 256 + which * 128
                    for ci in range(CT):
                        nc.tensor.matmul(
                            ps[:, :],
                            lhsT=wt[:, ci, w0:w0 + 128],
                            rhs=xtt[:, ci, :],
                            start=(ci == 0), stop=(ci == CT - 1))
                    if which == 0:
                        nc.vector.tensor_scalar_add(
                            qtt[:, g, :], ps[:, :], bcc[:, 2 * g:2 * g + 1])
                    else:
                        nc.vector.tensor_scalar_add(
                            kT[:, g, tj * 512:(tj + 1) * 512], ps[:, :],
                            bcc[:, 2 * g + 1:2 * g + 2])

            def v_chunk(tj):
                xtt = xts[tj]
                for tl in range(4):
                    tt = tj * 4 + tl
                    ps = shpsum.tile([128, 512], f32, name="ps", tag="ps")
                    for ci in range(CT):
                        nc.tensor.matmul(
                            ps[:, :],
                            lhsT=xtt[:, ci, tl * 128:(tl + 1) * 128],
                            rhs=wt[:, ci, 2 * OC:3 * OC],
                            start=(ci == 0), stop=(ci == CT - 1))
                    nc.vector.tensor_add(vA[:, tt, :, 0:D], ps[:, :],
                                         btrB[:, :])

            def s_phase(J, hp):
                """S^T + exp + causal mask for head pair hp, tq chunk J."""
                ni = 4 * J + 4
                slots = []
                qtt = qts[J]
                for i in range(ni):
                    c0 = max(0, (i - 4 * J) * 128)
                    ptt = ptp.tile([128, 2, 512], bf16, name="pt", tag="pt")
                    slots.append(ptt)
                    sp = spsum.tile([128, 2, 512], f32, name="sp", tag="sp")
                    for hc in range(2):
                        kp = hc * 64
                        nc.tensor.matmul(
                            sp[:, hc, c0:512],
                            lhsT=kT[kp:kp + 64, hp, i * 128:(i + 1) * 128],
                            rhs=qtt[kp:kp + 64, hp, c0:512],
                            start=True, stop=True)
                    nc.scalar.activation(
                        ptt[:, 0:2, c0:512], sp[:, 0:2, c0:512],
                        EXP, scale=0.125)
                    if i >= 4 * J:                         # diagonal tile
                        for hc in range(2):
                            nc.gpsimd.tensor_mul(
                                ptt[:, hc, c0:c0 + 128],
                                ptt[:, hc, c0:c0 + 128],
                                ut[:, :])
                pts[(J, hp)] = slots

            def pv_phase(J, hp, last=False):
                """P@v, denominators, transposes, normalize for (J, hp)."""
                ni = 4 * J + 4
                slots = pts.pop((J, hp))
                if hp == 0:
                    osbs[J] = [osbp.tile([128, OC], bf16, name=f"osb{J}_{jl}",
                                         tag=f"osb{jl}") for jl in range(4)]
                ytpair = ytp.tile([128, 512], bf16, name="yt", tag="yt")
                # dens of the head-pair PAIR (this hp and its partner) collect
                # on partitions {0,32,64,96} of one tile (bases must be
                # 32-aligned; the memset keeps the [97,128] transposes off
                # uninitialized bits).  The final pair (J=3) uses per-hp
                # dens instead so hp=2's normalization overlaps hp=3's P@v
                # and the end-of-kernel tail shortens.
                solo = (J == TJ - 1)
                if solo or hp % 2 == 0:
                    dn = denp.tile([97, 512], bf16, name="dn", tag="dn")
                    nc.gpsimd.memset(dn[0:65, :], 0.0)
                    dens[J] = dn
                else:
                    dn = dens[J]
                for hc in range(2):
                    h = 2 * hp + hc
                    psv = shpsum.tile([128, 512], f32, name="psv", tag="ps")
                    for i in range(ni):
                        c0 = max(0, (i - 4 * J) * 128)
                        nc.tensor.matmul(
                            psv[0:65, c0:512],
                            lhsT=vA[:, i, h, :],
                            rhs=slots[i][:, hc, c0:512],
                            start=(i == 0), stop=(i == ni - 1),
                            skip_group_check=(c0 > 0))
                    nc.vector.tensor_copy(
                        ytpair[hc * 64:(hc + 1) * 64, :], psv[0:64, :])
                    r = 32 * (hc if solo else 2 * (hp % 2) + hc)
                    nc.vector.tensor_copy(dn[r:r + 1, :], psv[64:65, :])
                # y back to token-major (normalization deferred to odd hp)
                tps = tpsum.tile([128, 4, 128], bf16, name="tps", tag="tps")
                for jl in range(4):
                    nc.tensor.transpose(
                        tps[:, jl, :], ytpair[:, jl * 128:(jl + 1) * 128],
                        iden[:, :])
                if not solo and hp % 2 == 0:
                    dens[(J, "tps")] = tps
                    return
                # denominators -> token-major reciprocals [128, (row), (jl)]
                nr = 65 if solo else 97
                dtp = shpsum.tile([128, 4, 100], bf16, name="dtp", tag="ps")
                for jl in range(4):
                    nc.tensor.transpose(
                        dtp[:, jl, 0:nr],
                        dn[0:nr, jl * 128:(jl + 1) * 128], iden[0:nr, 0:nr])
                rc = rcp.tile([128, 4, 4], f32, name="rc", tag="rc")
                for r4 in range(2 if solo else 4):
                    nc.vector.reciprocal(
                        rc[:, r4, :], dtp[:, :, 32 * r4:32 * r4 + 1])
                if solo:
                    pairs = ((hp, tps),)
                else:
                    pairs = ((hp - 1, dens.pop((J, "tps"))), (hp, tps))
                for hq, tq in pairs:
                    for jl in range(4):
                        for hc in range(2):
                            nc.vector.tensor_scalar_mul(
                                osbs[J][jl][:, hq * 128 + hc * 64:
                                            hq * 128 + (hc + 1) * 64],
                                tq[:, jl, hc * 64:(hc + 1) * 64],
                                rc[:, (hc if solo else
                                       2 * (hq % 2) + hc), jl:jl + 1])
                if last:
                    for jl in range(4):
                        r0 = (4 * J + jl) * 128
                        eng = nc.sync if jl % 2 == 0 else nc.gpsimd
                        eng.dma_start(out_d[r0:r0 + 128, :],
                                      osbs[J][jl][:, :])
                    del osbs[J]

            # ---- emission schedule ----
            # S^T/exp runs two head-pairs ahead of P@v (pt pool rotation is
            # deadlock-free at this distance with 32 bufs); QKV chunk J+1 is
            # emitted inside attention chunk J.
            load_chunk(0)

            def load_w(g):
                nc.sync.dma_start(wt[:, 0:4, g * 256:(g + 1) * 256],
                                  wt_d[0:512, g * 256:(g + 1) * 256])
                nc.sync.dma_start(wt[:, 4:8, g * 256:(g + 1) * 256],
                                  wt_d[512:1024, g * 256:(g + 1) * 256])

            nc.sync.dma_start(bcc[:, :], bcc_d[:, :])
            qk_od(0, 0); load_w(1); s_phase(0, 0)
            qk_od(0, 1); load_w(2); s_phase(0, 1)
            load_w(3)
            nc.sync.dma_start(wt[:, 0:4, 2 * OC:3 * OC],
                              wt_d[0:512, 2 * OC:3 * OC])
            nc.sync.dma_start(wt[:, 4:8, 2 * OC:3 * OC],
                              wt_d[512:1024, 2 * OC:3 * OC])
            nc.sync.dma_start(btr[:, :], btr_d[:, :])
            nc.gpsimd.partition_broadcast(btrB[:, :], btr[0:1, :])
            qk_od(0, 2); s_phase(0, 2)
            qk_od(0, 3)
            v_chunk(0)

            # S^T runs THREE head-pair phases ahead of P@v (pt pool rotation
            # is deadlock-free at this distance with exactly 48 bufs)
            sq = iter([(J2, h2) for J2 in range(TJ) for h2 in range(4)][3:])

            def next_s():
                nxt = next(sq, None)
                if nxt is not None:
                    s_phase(*nxt)

            for J in range(TJ):
                nj = J + 1
                pv_phase(J, 0); next_s()
                if nj < TJ:
                    load_chunk(nj)
                    qk_od(nj, 0); qk_od(nj, 1)
                pv_phase(J, 1); next_s()
                if nj < TJ:
                    qk_od(nj, 2); qk_od(nj, 3)
                pv_phase(J, 2); next_s()
                pv_phase(J, 3, last=True); next_s()
                if nj < TJ:
                    v_chunk(nj)

    nc.finalize()
    return nc


def _prep_inputs(x, W, b):
    """Build per-core input maps (host-side sharding + layout prep)."""
    in_maps = []
    for core in range(NCORES):
        bi, g = core // 2, core % 2
        h0 = g * HPC
        # weight rows, interleaved [q0,k0,q1,k1,q2,k2,q3,k3,v] by 128-row
        # o-tiles (o-tile g covers heads h0+2g, h0+2g+1)
        blocks = []
        for gg in range(4):
            r = (h0 + 2 * gg) * D
            blocks.append(np.arange(r, r + 128))           # q o-tile gg
            blocks.append(np.arange(C + r, C + r + 128))   # k o-tile gg
        blocks.append(np.arange(2 * C + h0 * D, 2 * C + h0 * D + OC))  # v
        rows = np.concatenate(blocks)
        Wc = W[rows, :]                                    # [1536, 1024]
        bcc = np.empty((128, 8), dtype=np.float32)
        for gg in range(4):
            r = (h0 + 2 * gg) * D
            bcc[:, 2 * gg] = b[r:r + 128]
            bcc[:, 2 * gg + 1] = b[C + r:C + r + 128]
        btr = b[2 * C + h0 * D:2 * C + h0 * D + OC]
        in_maps.append({
            "xt": np.ascontiguousarray(x[bi].T).astype(ml_dtypes.bfloat16),
            "wt": np.ascontiguousarray(Wc.T).astype(ml_dtypes.bfloat16),
            "bcc": bcc,
            "btr": btr.reshape(1, -1).astype(ml_dtypes.bfloat16),
        })
    return in_maps


def kernel(x, W, b):
    from concourse.bass_utils import run_bass_kernel_spmd

    if "nc" not in _cache:
        _cache["nc"] = _build_bass()
    nc = _cache["nc"]
    in_maps = _prep_inputs(np.asarray(x), np.asarray(W), np.asarray(b))
    res = run_bass_kernel_spmd(nc, in_maps, core_ids=list(range(NCORES)))
    out = np.empty((B, T, C), dtype=np.float32)
    for core in range(NCORES):
        bi, g = core // 2, core % 2
        out[bi][:, g * OC:(g + 1) * OC] = np.asarray(
            res.results[core]["out"], dtype=np.float32)
    return out


# revision 40
# speedup vs baseline: 1.0056x; 1.0056x over previous
"""Causal multi-head attention (QKV proj + 16-head causal attention) on 8 TRN2 cores.

Problem: x [4, 2048, 1024], W [3072, 1024], b [3072] -> out [4, 2048, 1024].
H=16 heads, D=64. Sharding: core c = (batch b = c // 2, head-group g = c % 2);
each core computes batch b, heads g*8 .. g*8+8, producing out[b][:, g*512:(g+1)*512].
No cross-core communication needed.

Device kernel (per core), all matmuls bf16 with f32 PSUM accumulation.
TensorE (~215us busy) and ScalarE (exp over the ~18M live logits, ~158us)
are the co-bottlenecks; the kernel software-pipelines them:
  - QKV projection chunked by 512-token groups. Q/K bias folded into the
    PSUM->SBUF cast via per-partition tensor_scalar_add; V bias added during
    the V cast via a tensor_tensor add against a GpSimd partition_broadcast
    of the bias row (no bias matmuls at all). V lands via one strided CAST
    per t-tile in vA [128, 16, 8, 65]; only vA's ones columns (softmax-
    denominator trick) are memset, keeping GpSimd startup off the
    critical path. Ten identity transposes at t~8us warm the PE clock (HAM)
    so the first projection matmuls run at 2.4 GHz.
  - Attention per (tq-chunk J of 512, head pair hp): S^T tiles [tk=128, tq]
    = kT.T @ qT with the head dim contracting on 64 partitions and both heads
    of the pair issued adjacently (concurrent sub-array execution); exp on
    ScalarE over 2-slot PSUM groups trimmed to the causally-live column
    range (saves ~30us of ScalarE vs full-width); a single 128x128
    upper-triangular mask multiply (on GpSimd, otherwise idle) for diagonal
    tiles only.
  - P@v v-stationary: y^T[65, tq] = sum_i [v_i|1].T @ P^T_i (row 64 = softmax
    denominator). Both heads' y^T pack into one [128, 512] tile via
    partition-shifted DVE casts, then 4 128x128 PE transposes per head pair
    restore token-major layout. Denominators of a head-pair PAIR collect on
    partitions {0,32,64,96} of one [97, 512] tile (DVE partition bases must
    be 32-aligned; memset first so the transposes never touch uninitialized
    bits) -> 4 [97,128] transposes + 4 reciprocals per TWO head pairs, then
    per-partition tensor_scalar_mul normalization straight out of transpose
    PSUM into the output staging tiles.
  - Emission keeps S^T/exp THREE head-pair phases ahead of P@v so ScalarE
    never starves and P@v never waits on exp, with QKV chunk J+1 emitted
    inside attention chunk J. P^T tiles live in a 48-deep pool of per-i-tile
    [128, 2, 512] buffers whose rotation order is deadlock-free at this
    lookahead by construction. Output stages and stores in bf16 (error
    budget 6x headroom), halving output DMA traffic, with the four chunk
    stores split across the Sync and GpSimd DMA queues.
    Input DMAs keep the [512-row] -> [128, 4, *] shape: that split maps
    src row = ct*128 + p; other shapes pair dims differently (verified).
Measured: ~242-244 us NEFF exec (baseline 324 us), rel err 3.4e-3.
"""

import numpy as np
import ml_dtypes

B, T, C = 4, 2048, 1024
H, D = 16, 64
HPC = 8            # heads per core
OC = HPC * D       # 512 output cols per core
NCORES = 8

_cache = {}


def _build_bass():
    import concourse.mybir as mybir
    import concourse.tile as tile
    from concourse import bacc
    from concourse.masks import make_identity, make_upper_triangular

    f32 = mybir.dt.float32
    bf16 = mybir.dt.bfloat16
    EXP = mybir.ActivationFunctionType.Exp

    nc = bacc.Bacc(None)
    xt_d = nc.declare_dram_parameter("xt", [C, T], bf16, isOutput=False)
    wt_d = nc.declare_dram_parameter("wt", [C, 3 * OC], bf16, isOutput=False)
    bcc_d = nc.declare_dram_parameter("bcc", [128, 8], f32, isOutput=False)
    btr_d = nc.declare_dram_parameter("btr", [1, OC], bf16, isOutput=False)
    out_d = nc.declare_dram_parameter("out", [T, OC], bf16, isOutput=True)

    CT = C // 128     # 8 c-tiles
    TT = T // 128     # 16 t-tiles
    TJ = T // 512     # 4 big t-chunks

    with tile.TileContext(nc) as tc:
        with (
            tc.tile_pool(name="persist", bufs=1) as persist,
            tc.tile_pool(name="xtp", bufs=2) as xtp,
            tc.tile_pool(name="qtp", bufs=2) as qtp,
            tc.tile_pool(name="ptp", bufs=48) as ptp,
            tc.tile_pool(name="ytp", bufs=2) as ytp,
            tc.tile_pool(name="denp", bufs=2) as denp,
            tc.tile_pool(name="rcp", bufs=2) as rcp,
            tc.tile_pool(name="osbp", bufs=4) as osbp,
            tc.tile_pool(name="spsum", bufs=2, space="PSUM") as spsum,
            tc.tile_pool(name="shpsum", bufs=2, space="PSUM") as shpsum,
            tc.tile_pool(name="tpsum", bufs=2, space="PSUM") as tpsum,
        ):
            # ---- persistent SBUF tensors ----
            wt = persist.tile([128, CT, 3 * OC], bf16)     # [c%128, c//128, o]
            kT = persist.tile([128, OC // 128, T], bf16)   # [o%128, o//128, t]
            vA = persist.tile([128, TT, HPC, D + 1], bf16)  # v + ones col
            bcc = persist.tile([128, 8], f32)              # Q/K bias, col=o-tile
            btr = persist.tile([1, OC], bf16)              # V bias row
            btrB = persist.tile([128, OC], bf16)           # V bias bcast to all p
            ut = persist.tile([128, 128], bf16)            # upper-tri (incl diag)
            iden = persist.tile([128, 128], bf16)

            # early DMAs: first Q/K weight block + tokens chunk 0 + Q/K bias
            nc.sync.dma_start(wt[:, 0:4, 0:256], wt_d[0:512, 0:256])
            nc.gpsimd.dma_start(wt[:, 4:8, 0:256], wt_d[512:1024, 0:256])

            make_identity(nc, iden[:, :])
            # PE clock (HAM) warmup: identity transposes keep the PE busy from
            # ~1us so the first real matmuls run at 2.4 GHz instead of 1.2
            warm = tpsum.tile([128, 4, 128], bf16, name="warm", tag="tps")
            for k in range(10):
                nc.tensor.transpose(warm[:, k % 4, :], iden[:, :], iden[:, :])
            make_upper_triangular(nc, ut[:, :], val=1.0, diag=True)
            for h in range(HPC):                           # vA ones columns
                nc.gpsimd.memset(vA[:, :, h, D:D + 1], 1.0)

            xts = [None] * TJ
            qts = [None] * TJ
            osbs = {}
            pts = {}

            def load_chunk(tj):
                xts[tj] = xtp.tile([128, CT, 512], bf16, name=f"xt{tj}", tag="xt")
                nc.sync.dma_start(xts[tj][:, 0:4, :],
                                  xt_d[0:512, tj * 512:(tj + 1) * 512])
                nc.gpsimd.dma_start(xts[tj][:, 4:8, :],
                                  xt_d[512:1024, tj * 512:(tj + 1) * 512])
                qts[tj] = qtp.tile([128, 4, 512], bf16, name=f"qt{tj}", tag="qt")

            dens = {}

            def qk_od(tj, g):
                """Project q and k o-tile g for token chunk tj."""
                xtt, qtt = xts[tj], qts[tj]
                for which in range(2):                     # 0 = q, 1 = k
                    ps = shpsum.tile([128, 512], f32, name="ps", tag="ps")
                    w0 = g * 256 + which * 128
                    for ci in range(CT):
                        nc.tensor.matmul(
                            ps[:, :],
                            lhsT=wt[:, ci, w0:w0 + 128],
                            rhs=xtt[:, ci, :],
                            start=(ci == 0), stop=(ci == CT - 1))
                    if which == 0:
                        nc.vector.tensor_scalar_add(
                            qtt[:, g, :], ps[:, :], bcc[:, 2 * g:2 * g + 1])
                    else:
                        nc.vector.tensor_scalar_add(
                            kT[:, g, tj * 512:(tj + 1) * 512], ps[:, :],
                            bcc[:, 2 * g + 1:2 * g + 2])

            def v_chunk(tj):
                xtt = xts[tj]
                for tl in range(4):
                    tt = tj * 4 + tl
                    ps = shpsum.tile([128, 512], f32, name="ps", tag="ps")
                    for ci in range(CT):
                        nc.tensor.matmul(
                            ps[:, :],
                            lhsT=xtt[:, ci, tl * 128:(tl + 1) * 128],
                            rhs=wt[:, ci, 2 * OC:3 * OC],
                            start=(ci == 0), stop=(ci == CT - 1))
                    nc.vector.tensor_add(vA[:, tt, :, 0:D], ps[:, :],
                                         btrB[:, :])

            def s_phase(J, hp):
                """S^T + exp + causal mask for head pair hp, tq chunk J."""
                ni = 4 * J + 4
                slots = []
                qtt = qts[J]
                for i in range(ni):
                    c0 = max(0, (i - 4 * J) * 128)
                    ptt = ptp.tile([128, 2, 512], bf16, name="pt", tag="pt")
                    slots.append(ptt)
                    sp = spsum.tile([128, 2, 512], f32, name="sp", tag="sp")
                    for hc in range(2):
                        kp = hc * 64
                        nc.tensor.matmul(
                            sp[:, hc, c0:512],
                            lhsT=kT[kp:kp + 64, hp, i * 128:(i + 1) * 128],
                            rhs=qtt[kp:kp + 64, hp, c0:512],
                            start=True, stop=True)
                    nc.scalar.activation(
                        ptt[:, 0:2, c0:512], sp[:, 0:2, c0:512],
                        EXP, scale=0.125)
                    if i >= 4 * J:                         # diagonal tile
                        for hc in range(2):
                            nc.gpsimd.tensor_mul(
                                ptt[:, hc, c0:c0 + 128],
                                ptt[:, hc, c0:c0 + 128],
                                ut[:, :])
                pts[(J, hp)] = slots

            def pv_phase(J, hp, last=False):
                """P@v, denominators, transposes, normalize for (J, hp)."""
                ni = 4 * J + 4
                slots = pts.pop((J, hp))
                if hp == 0:
                    osbs[J] = [osbp.tile([128, OC], bf16, name=f"osb{J}_{jl}",
                                         tag=f"osb{jl}") for jl in range(4)]
                ytpair = ytp.tile([128, 512], bf16, name="yt", tag="yt")
                # dens of the head-pair PAIR (this hp and its partner) collect
                # on partitions {0,32,64,96} of one tile (bases must be
                # 32-aligned; the memset keeps the [97,128] transposes off
                # uninitialized bits)
                if hp % 2 == 0:
                    dn = denp.tile([97, 512], bf16, name="dn", tag="dn")
                    nc.gpsimd.memset(dn[:, :], 0.0)
                    dens[J] = dn
                else:
                    dn = dens[J]
                for hc in range(2):
                    h = 2 * hp + hc
                    psv = shpsum.tile([128, 512], f32, name="psv", tag="ps")
                    for i in range(ni):
                        c0 = max(0, (i - 4 * J) * 128)
                        nc.tensor.matmul(
                            psv[0:65, c0:512],
                            lhsT=vA[:, i, h, :],
                            rhs=slots[i][:, hc, c0:512],
                            start=(i == 0), stop=(i == ni - 1),
                            skip_group_check=(c0 > 0))
                    nc.vector.tensor_copy(
                        ytpair[hc * 64:(hc + 1) * 64, :], psv[0:64, :])
                    r = 32 * (2 * (hp % 2) + hc)
                    nc.vector.tensor_copy(dn[r:r + 1, :], psv[64:65, :])
                # y back to token-major (normalization deferred to odd hp)
                tps = tpsum.tile([128, 4, 128], bf16, name="tps", tag="tps")
                for jl in range(4):
                    nc.tensor.transpose(
                        tps[:, jl, :], ytpair[:, jl * 128:(jl + 1) * 128],
                        iden[:, :])
                if hp % 2 == 0:
                    dens[(J, "tps")] = tps
                    return
                # denominators -> token-major reciprocals [128, (row), (jl)]
                dtp = shpsum.tile([128, 4, 100], bf16, name="dtp", tag="ps")
                for jl in range(4):
                    nc.tensor.transpose(
                        dtp[:, jl, 0:97],
                        dn[:, jl * 128:(jl + 1) * 128], iden[0:97, 0:97])
                rc = rcp.tile([128, 4, 4], f32, name="rc", tag="rc")
                for r4 in range(4):
                    nc.vector.reciprocal(
                        rc[:, r4, :], dtp[:, :, 32 * r4:32 * r4 + 1])
                tps_e = dens.pop((J, "tps"))
                for hq, tq in ((hp - 1, tps_e), (hp, tps)):
                    for jl in range(4):
                        for hc in range(2):
                            nc.vector.tensor_scalar_mul(
                                osbs[J][jl][:, hq * 128 + hc * 64:
                                            hq * 128 + (hc + 1) * 64],
                                tq[:, jl, hc * 64:(hc + 1) * 64],
                                rc[:, 2 * (hq % 2) + hc, jl:jl + 1])
                if last:
                    for jl in range(4):
                        r0 = (4 * J + jl) * 128
                        eng = nc.sync if jl % 2 == 0 else nc.gpsimd
                        eng.dma_start(out_d[r0:r0 + 128, :],
                                      osbs[J][jl][:, :])
                    del osbs[J]

            # ---- emission schedule ----
            # S^T/exp runs two head-pairs ahead of P@v (pt pool rotation is
            # deadlock-free at this distance with 32 bufs); QKV chunk J+1 is
            # emitted inside attention chunk J.
            load_chunk(0)

            def load_w(g):
                nc.sync.dma_start(wt[:, 0:4, g * 256:(g + 1) * 256],
                                  wt_d[0:512, g * 256:(g + 1) * 256])
                nc.sync.dma_start(wt[:, 4:8, g * 256:(g + 1) * 256],
                                  wt_d[512:1024, g * 256:(g + 1) * 256])

            nc.sync.dma_start(bcc[:, :], bcc_d[:, :])
            qk_od(0, 0); load_w(1); s_phase(0, 0)
            qk_od(0, 1); load_w(2); s_phase(0, 1)
            load_w(3)
            nc.sync.dma_start(wt[:, 0:4, 2 * OC:3 * OC],
                              wt_d[0:512, 2 * OC:3 * OC])
            nc.sync.dma_start(wt[:, 4:8, 2 * OC:3 * OC],
                              wt_d[512:1024, 2 * OC:3 * OC])
            nc.sync.dma_start(btr[:, :], btr_d[:, :])
            nc.gpsimd.partition_broadcast(btrB[:, :], btr[0:1, :])
            qk_od(0, 2); s_phase(0, 2)
            qk_od(0, 3)
            v_chunk(0)

            # S^T runs THREE head-pair phases ahead of P@v (pt pool rotation
            # is deadlock-free at this distance with exactly 48 bufs)
            sq = iter([(J2, h2) for J2 in range(TJ) for h2 in range(4)][3:])

            def next_s():
                nxt = next(sq, None)
                if nxt is not None:
                    s_phase(*nxt)

            for J in range(TJ):
                nj = J + 1
                pv_phase(J, 0); next_s()
                if nj < TJ:
                    load_chunk(nj)
                    qk_od(nj, 0); qk_od(nj, 1)
                pv_phase(J, 1); next_s()
                if nj < TJ:
                    qk_od(nj, 2); qk_od(nj, 3)
                pv_phase(J, 2); next_s()
                pv_phase(J, 3, last=True); next_s()
                if nj < TJ:
                    v_chunk(nj)

    nc.finalize()
    return nc


def _prep_inputs(x, W, b):
    """Build per-core input maps (host-side sharding + layout prep)."""
    in_maps = []
    for core in range(NCORES):
        bi, g = core // 2, core % 2
        h0 = g * HPC
        # weight rows, interleaved [q0,k0,q1,k1,q2,k2,q3,k3,v] by 128-row
        # o-tiles (o-tile g covers heads h0+2g, h0+2g+1)
        blocks = []
        for gg in range(4):
            r = (h0 + 2 * gg) * D
            blocks.append(np.arange(r, r + 128))           # q o-tile gg
            blocks.append(np.arange(C + r, C + r + 128))   # k o-tile gg
        blocks.append(np.arange(2 * C + h0 * D, 2 * C + h0 * D + OC))  # v
        rows = np.concatenate(blocks)
        Wc = W[rows, :]                                    # [1536, 1024]
        bcc = np.empty((128, 8), dtype=np.float32)
        for gg in range(4):
            r = (h0 + 2 * gg) * D
            bcc[:, 2 * gg] = b[r:r + 128]
            bcc[:, 2 * gg + 1] = b[C + r:C + r + 128]
        btr = b[2 * C + h0 * D:2 * C + h0 * D + OC]
        in_maps.append({
            "xt": np.ascontiguousarray(x[bi].T).astype(ml_dtypes.bfloat16),
            "wt": np.ascontiguousarray(Wc.T).astype(ml_dtypes.bfloat16),
            "bcc": bcc,
            "btr": btr.reshape(1, -1).astype(ml_dtypes.bfloat16),
        })
    return in_maps


def kernel(x, W, b):
    from concourse.bass_utils import run_bass_kernel_spmd

    if "nc" not in _cache:
        _cache["nc"] = _build_bass()
    nc = _cache["nc"]
    in_maps = _prep_inputs(np.asarray(x), np.asarray(W), np.asarray(b))
    res = run_bass_kernel_spmd(nc, in_maps, core_ids=list(range(NCORES)))
    out = np.empty((B, T, C), dtype=np.float32)
    for core in range(NCORES):
        bi, g = core // 2, core % 2
        out[bi][:, g * OC:(g + 1) * OC] = np.asarray(
            res.results[core]["out"], dtype=np.float32)
    return out
